# revision 17
# baseline (speedup 1.0000x reference)
"""Multi-head causal attention on 8 Trainium2 NeuronCores.

Sharding: data-parallel over batch (4) x tensor-parallel over heads (2 groups
of 8 heads). Each core computes a partial output [T, C] for one batch element
using its 8 heads; the host sums the two partials per batch element (the
"all-reduce after out_proj" done during unshard).

Single software-pipelined emission schedule (per core):
  proj(0) | attn(0)+fill[proj(1)] | attn(1)+fill[proj(2), tail(0)] |
  attn(2)+fill[proj(3), tail(1)] | attn(3)+fill[tail(2)] | tail(3)
where tail(j) = softmax-normalize + output projection + DMA for query
chunk j.  "fill" pieces are small PE matmul groups emitted between a
score matmul and its dependent AV matmul so the PE never idles while
the Scalar engine computes exp (the scalar exp stream, ~125us, is the
co-bottleneck with the PE's ~230us of matmul columns).

Per-core layouts (no on-device transposes):
  QT/KT per chunk t: [128, 4, 512] bf16  (ch%128, ch//128, tok)
  V_aug per chunk:   [128, 4, 8, 65] bf16 (key%128, tb, head, d; col 64 = 1)
  scores per (j,p,kb): PSUM [128, 2w]; w trimmed to 512-128m on the
    diagonal blocks (m = kb-4j >= 0), only the first 128 columns of a
    diagonal block need the triangular mask.
  softmax denominator rows staged in dens[32j, slot] (engine APs may only
    start at partitions {0,32,64,96}); per-j DMA compacts to [8, 512],
    reciprocal_approx_fast, then a small sel-matmul broadcasts 1/denom
    back to 64 partitions for the normalize multiply.
"""

import numpy as np
import ml_dtypes

_BF = ml_dtypes.bfloat16

import concourse.bass as bass
import concourse.bacc as bacc
import concourse.mybir as mybir
import concourse.tile as tile
from concourse import bass_utils

F32 = mybir.dt.float32
F32R = mybir.dt.float32r
BF16 = mybir.dt.bfloat16

B, T, C = 4, 2048, 1024
H, Dh = 16, 64
G = 2                 # head groups (tensor parallel)
HPG = H // G          # heads per group = 8
GC = HPG * Dh         # group channels = 512
N_CORES = 8
TC = 512              # token chunk
KB = 128              # key block
N_TC = T // TC        # 4
N_CC = C // 128       # contraction chunks over C = 8
N_GCB = GC // 128     # chan blocks in a group = 4


class _Fillers:
    """Queue of small emission pieces drained between score and AV matmuls."""

    def __init__(self):
        self.q = []
        self.i = 0

    def add(self, fn):
        self.q.append(fn)

    def remaining(self):
        return len(self.q) - self.i

    def drain(self, n):
        stop = min(self.i + n, len(self.q))
        while self.i < stop:
            self.q[self.i]()
            self.i += 1

    def drain_all(self):
        self.drain(len(self.q))


def build_program():
    nc = bacc.Bacc("TRN2", target_bir_lowering=False, debug=False)

    xT = nc.dram_tensor("xT", [C, T], BF16, kind="ExternalInput").ap()
    wq = nc.dram_tensor("wq", [C, GC], BF16, kind="ExternalInput").ap()
    wk = nc.dram_tensor("wk", [C, GC], BF16, kind="ExternalInput").ap()
    wv = nc.dram_tensor("wv", [C, GC], BF16, kind="ExternalInput").ap()
    wo = nc.dram_tensor("wo", [GC, C], BF16, kind="ExternalInput").ap()
    mask_in = nc.dram_tensor("mask", [KB, KB], BF16, kind="ExternalInput").ap()
    sel_in = nc.dram_tensor("sel", [8, 8 * Dh], BF16, kind="ExternalInput").ap()
    out = nc.dram_tensor("out", [T, C], BF16, kind="ExternalOutput").ap()

    with tile.TileContext(nc) as tc:
        with (
            tc.tile_pool(name="persist", bufs=1) as pp,
            tc.tile_pool(name="xt_pool", bufs=2) as xp,
            tc.tile_pool(name="pr_pool", bufs=4) as prp,
            tc.tile_pool(name="aot_pool", bufs=2) as aop,
            tc.tile_pool(name="ot_pool", bufs=2) as otp,
            tc.tile_pool(name="dc_pool", bufs=6) as dcp,
            tc.tile_pool(name="pj_psum", bufs=2, space="PSUM") as pjp,
            tc.tile_pool(name="sc_psum", bufs=2, space="PSUM") as scp,
            tc.tile_pool(name="av_psum", bufs=2, space="PSUM") as avp,
        ):
            wqs = pp.tile([128, N_CC, GC], BF16, tag="wq")
            wks = pp.tile([128, N_CC, GC], BF16, tag="wk")
            wvs = pp.tile([128, N_CC, GC], BF16, tag="wv")
            wos = pp.tile([128, N_GCB, C], BF16, tag="wo")
            msk = pp.tile([KB, KB], BF16, tag="msk")
            sel8 = pp.tile([8, 8 * Dh], BF16, tag="sel")
            dens = pp.tile([97, 8, TC], F32, tag="dens")
            qts = [pp.tile([128, N_GCB, TC], BF16, tag=f"qt{t}", name=f"qt{t}")
                   for t in range(N_TC)]
            kts = [pp.tile([128, N_GCB, TC], BF16, tag=f"kt{t}", name=f"kt{t}")
                   for t in range(N_TC)]
            vgs = [
                pp.tile([128, 4, HPG, Dh + 1], BF16, tag=f"vg{t}", name=f"vg{t}")
                for t in range(N_TC)
            ]

            # input DMAs, most-urgent first (proj(0) QT needs wq + xt(0));
            # interleave wq/xt chunks so the first QT group starts early
            xt0 = xp.tile([128, N_CC, TC], BF16, tag="xt", name="xt")
            for kc in range(N_CC):
                nc.scalar.dma_start(wqs[:, kc, :], wq[kc * 128:(kc + 1) * 128, :])
                nc.sync.dma_start(
                    xt0[:, kc, :], xT[kc * 128:(kc + 1) * 128, 0:TC]
                )
            for kc in range(N_CC):
                nc.scalar.dma_start(wks[:, kc, :], wk[kc * 128:(kc + 1) * 128, :])
            for kc in range(N_CC):
                nc.sync.dma_start(wvs[:, kc, :], wv[kc * 128:(kc + 1) * 128, :])
            nc.sync.dma_start(msk[:], mask_in)
            for cb in range(N_GCB):
                nc.sync.dma_start(wos[:, cb, :], wo[cb * 128:(cb + 1) * 128, :])
            nc.sync.dma_start(sel8[:], sel_in)
            for t in range(N_TC):
                nc.vector.memset(vgs[t][:, :, :, Dh:], 1.0)

            def emit_proj_pieces(t, F, xt, parts=("q", "k", "v")):
                """Append proj-chunk-t pieces (2 matmuls each) to F."""
                srcs = []
                if "q" in parts:
                    srcs.append((wqs, qts))
                if "k" in parts:
                    srcs.append((wks, kts))
                for w_s, dsts in srcs:
                    for oc in range(N_GCB):
                        box = [None]

                        def qk_piece(pc, oc=oc, w_s=w_s, dsts=dsts, box=box,
                                     t=t, xt=xt):
                            if pc == 0:
                                box[0] = pjp.tile([128, TC], F32, tag="pj", name="pj")
                            for kc in (2 * pc, 2 * pc + 1):
                                nc.tensor.matmul(
                                    box[0][:],
                                    w_s[:, kc, oc * 128:(oc + 1) * 128],
                                    xt[:, kc, :],
                                    start=(kc == 0),
                                    stop=(kc == N_CC - 1),
                                )
                            if pc == 3:
                                nc.vector.tensor_copy(dsts[t][:, oc, :], box[0][:])

                        for pc in range(4):
                            F.add(lambda pc=pc, f=qk_piece: f(pc))
                for tb in (range(4) if "v" in parts else ()):
                    box = [None]

                    def v_piece(pc, tb=tb, box=box, t=t, xt=xt):
                        if pc == 0:
                            box[0] = pjp.tile([128, GC], F32, tag="pj", name="pj")
                        for kc in (2 * pc, 2 * pc + 1):
                            nc.tensor.matmul(
                                box[0][:],
                                xt[:, kc, tb * 128:(tb + 1) * 128],
                                wvs[:, kc, :],
                                start=(kc == 0),
                                stop=(kc == N_CC - 1),
                            )
                        if pc == 3:
                            nc.vector.tensor_copy(
                                vgs[t][:, tb, :, :Dh],
                                box[0].rearrange("p (h d) -> p h d", h=HPG),
                            )

                    for pc in range(4):
                        F.add(lambda pc=pc, f=v_piece: f(pc))

            def emit_bcnorm(j, p4, aot_j, rc2):
                """Broadcast 1/denom for head-pair p4 and normalize aot."""
                for half in range(2):
                    bc = pjp.tile([Dh, TC], F32, tag="pj", name="bc")
                    nc.tensor.matmul(
                        bc[:],
                        sel8[0:2, half * Dh:(half + 1) * Dh],
                        rc2[:],
                        start=True, stop=True,
                    )
                    slc = aot_j[half * Dh:(half + 1) * Dh, p4, :]
                    nc.vector.tensor_mul(slc, slc, bc[:])

            def emit_tail_pieces(j, F, aot_j, last_bc=None):
                """Project normalized aot(j), DMA out rows 512j..512j+512.

                last_bc: deferred bc+norm closure for head-pair 3 (its
                reciprocal chain finishes just after attn(j) ends)."""
                if last_bc is not None:
                    F.add(last_bc)
                for tbl in range(4):
                    obox = [None]
                    for oc in range(2):
                        def o_piece(oc=oc, tbl=tbl, obox=obox, j=j,
                                    aot_j=aot_j):
                            if oc == 0:
                                obox[0] = otp.tile([128, C], BF16, tag="ot", name="ot")
                            ps = pjp.tile([128, TC], F32, tag="pj", name="pj")
                            for cc in range(N_GCB):
                                nc.tensor.matmul(
                                    ps[:],
                                    aot_j[:, cc, tbl * 128:(tbl + 1) * 128],
                                    wos[:, cc, oc * TC:(oc + 1) * TC],
                                    start=(cc == 0),
                                    stop=(cc == N_GCB - 1),
                                )
                            nc.vector.tensor_copy(
                                obox[0][:, oc * TC:(oc + 1) * TC], ps[:]
                            )
                            if oc == 1:
                                r0 = (4 * j + tbl) * 128
                                nc.sync.dma_start(
                                    out[r0:r0 + 128, :], obox[0][:]
                                )

                        F.add(o_piece)

            def emit_attn(j, F, Fu=None, u_deadline=10):
                """Attention for query chunk j; drains F between sc and av.

                AV matmuls run one kb behind the score matmuls (1-deep
                software pipeline) so the PE reaches av(kb) only after
                sc(kb+1)+fillers have covered the exp(kb) latency.  Fu is
                an urgent queue (KT/V projections of chunk 3) fully
                drained before iteration u_deadline."""
                aot_j = aop.tile([128, N_GCB, TC], BF16, tag="aot", name="aot")
                nkb = 4 * j + 4
                iters = 4 * nkb
                it = 0
                rec_ps = []

                def emit_av(st):
                    kb, pr, w, off, tkb, ckb, p, pav = st
                    for half in range(2):
                        nc.tensor.matmul(
                            pav[half][:, off:TC],
                            vgs[tkb][:, ckb, 2 * p + half, :],
                            pr[:, half * TC:half * TC + w],
                            start=(kb == 0),
                            stop=(kb == nkb - 1),
                        )

                def emit_fin(p, pav, defer_aot=False):
                    aot_ops = []
                    if j < 2:
                        nc.scalar.copy(
                            dens[32 * j:32 * j + 1, 2 * p, :],
                            pav[0][Dh:Dh + 1, :],
                        )
                        nc.scalar.copy(aot_j[0:Dh, p, :], pav[0][:Dh, :])
                    else:
                        nc.vector.tensor_copy(
                            dens[32 * j:32 * j + 1, 2 * p, :],
                            pav[0][Dh:Dh + 1, :],
                        )
                        aot_ops.append(lambda: nc.vector.tensor_copy(
                            aot_j[0:Dh, p, :], pav[0][:Dh, :]))
                    nc.vector.tensor_copy(
                        dens[32 * j:32 * j + 1, 2 * p + 1, :],
                        pav[1][Dh:Dh + 1, :],
                    )
                    aot_ops.append(lambda: nc.vector.tensor_copy(
                        aot_j[Dh:2 * Dh, p, :], pav[1][:Dh, :]))
                    if not defer_aot:
                        for op in aot_ops:
                            op()
                        return None
                    return aot_ops

                for p in range(4):
                    av = [
                        avp.tile([Dh + 1, TC], F32, tag="av", name=f"av{i}")
                        for i in range(2)
                    ]
                    pend = None
                    for kb in range(nkb):
                        m = kb - 4 * j
                        w = TC if m < 0 else TC - m * 128
                        off = TC - w
                        tkb, ckb = kb // 4, kb % 4
                        # halves at bank-aligned offsets 0 / 512 (a matmul
                        # output may not cross a PSUM bank boundary)
                        sc = scp.tile([128, 2 * TC], F32, tag="sc", name="sc")
                        for half in range(2):
                            p0 = half * Dh
                            nc.tensor.matmul(
                                sc[:, half * TC:half * TC + w],
                                kts[tkb][p0:p0 + Dh, p, ckb * 128:(ckb + 1) * 128],
                                qts[j][p0:p0 + Dh, p, off:TC],
                                start=True, stop=True,
                            )
                        if Fu is not None and Fu.remaining():
                            if it < u_deadline:
                                Fu.drain(-(-Fu.remaining() // (u_deadline - it)))
                            else:
                                Fu.drain_all()
                        left = iters - it
                        F.drain(-(-F.remaining() // left))  # ceil
                        pr = prp.tile([128, 2 * TC], BF16, tag="pr", name="pr")
                        sc3 = sc.rearrange("p (b c) -> p b c", b=2)[:, :, 0:w]
                        pr3 = pr.rearrange("p (b c) -> p b c", b=2)[:, :, 0:w]
                        nc.scalar.activation(
                            pr3, sc3, mybir.ActivationFunctionType.Exp,
                        )
                        if m >= 0:
                            for half in range(2):
                                pslc = pr[:, half * TC:half * TC + KB]
                                nc.gpsimd.tensor_mul(pslc, pslc, msk[:])
                        if pend is not None:
                            emit_av(pend)
                        pend = (kb, pr, w, off, tkb, ckb, p, av)
                        it += 1
                    # bc+normalize of the previous pair: PE work that does
                    # not depend on this pair's last exp — covers its wait
                    if p >= 1:
                        emit_bcnorm(j, p - 1, aot_j, rec_ps[p - 1])
                    F.drain(2)
                    emit_av(pend)
                    deferred = emit_fin(p, av, defer_aot=(j == 3 and p == 3))
                    # per-pair denominator reciprocal; its ~3us chain
                    # completes during the next pair's kb loop
                    dc2 = dcp.tile([2, TC], F32, tag="dc", name="dc")
                    nc.sync.dma_start(
                        dc2[:], dens[32 * j:32 * j + 1, 2 * p:2 * p + 2, :]
                    )
                    rf2 = dcp.tile([2, TC], F32, tag="recf", name="recf")
                    nc.vector.reciprocal_approx_fast(rf2[:], dc2[:])
                    rc2 = dcp.tile([2, TC], BF16, tag="rec", name="rec")
                    nc.vector.tensor_copy(rc2[:], rf2[:])
                    rec_ps.append(rc2)
                    if deferred:
                        for op in deferred:
                            op()

                def last_bc(j=j, aot_j=aot_j, rc2=rec_ps[3]):
                    emit_bcnorm(j, 3, aot_j, rc2)
                return aot_j, last_bc

            # ---- pipelined emission ----
            F = _Fillers()
            emit_proj_pieces(0, F, xt0)
            F.drain_all()

            aots, recs = {}, {}
            xt3 = None
            for j in range(N_TC):
                F = _Fillers()
                Fu = None
                if j + 1 < N_TC:
                    xt = xp.tile([128, N_CC, TC], BF16, tag="xt", name="xt")
                    for kc in range(N_CC):
                        nc.sync.dma_start(
                            xt[:, kc, :],
                            xT[kc * 128:(kc + 1) * 128,
                               (j + 1) * TC:(j + 2) * TC],
                        )
                    if j + 1 == 3:
                        # KT/V of chunk 3 are first needed at attn(3) kb=12;
                        # defer them into attn(3)'s urgent queue so its thin
                        # filler stream gets more PE work
                        emit_proj_pieces(3, F, xt, parts=("q",))
                        xt3 = xt
                    else:
                        emit_proj_pieces(j + 1, F, xt)
                if j == 3:
                    Fu = _Fillers()
                    emit_proj_pieces(3, Fu, xt3, parts=("k", "v"))
                if j - 1 >= 0:
                    emit_tail_pieces(j - 1, F, aots[j - 1], recs[j - 1])
                aots[j], recs[j] = emit_attn(j, F, Fu)
                F.drain_all()
                if j == N_TC - 1:
                    recs[j]()
                    recs[j] = None

            F = _Fillers()
            emit_tail_pieces(N_TC - 1, F, aots[N_TC - 1])
            F.drain_all()

    nc.compile()
    return nc


_CACHE = {}


def _make_mask():
    m = np.zeros((KB, KB), np.float32)
    for dk in range(KB):
        m[dk, dk:] = 1.0
    return m.astype(_BF)


def _make_sel():
    s = np.zeros((8, 8 * Dh), np.float32)
    for i in range(8):
        s[i, i * Dh:(i + 1) * Dh] = 1.0
    return s.astype(_BF)


def make_in_maps(x, W_qkv, W_out):
    mask = _make_mask()
    sel = _make_sel()
    in_maps = []
    for core in range(N_CORES):
        b, g = divmod(core, G)
        cs = slice(g * GC, (g + 1) * GC)
        in_maps.append({
            "xT": np.ascontiguousarray(x[b].T).astype(_BF),
            # 1/sqrt(Dh) folded into wq
            "wq": np.ascontiguousarray(W_qkv[:, cs] * 0.125).astype(_BF),
            "wk": np.ascontiguousarray(
                W_qkv[:, C + g * GC:C + (g + 1) * GC]).astype(_BF),
            "wv": np.ascontiguousarray(
                W_qkv[:, 2 * C + g * GC:2 * C + (g + 1) * GC]).astype(_BF),
            "wo": np.ascontiguousarray(W_out[cs, :]).astype(_BF),
            "mask": mask,
            "sel": sel,
        })
    return in_maps


def kernel(x, W_qkv, W_out):
    x = np.ascontiguousarray(np.asarray(x, dtype=np.float32))
    W_qkv = np.asarray(W_qkv, dtype=np.float32)
    W_out = np.asarray(W_out, dtype=np.float32)

    if "nc" not in _CACHE:
        _CACHE["nc"] = build_program()
    nc = _CACHE["nc"]

    in_maps = make_in_maps(x, W_qkv, W_out)
    res = bass_utils.run_bass_kernel_spmd(nc, in_maps, core_ids=list(range(N_CORES)))

    out = np.empty((B, T, C), np.float32)
    for b in range(B):
        out[b] = res.results[G * b]["out"].astype(np.float32)
        for g in range(1, G):
            out[b] += res.results[G * b + g]["out"].astype(np.float32)
    return out


# revision 18
# speedup vs baseline: 1.0233x; 1.0233x over previous
"""Multi-head causal attention on 8 Trainium2 NeuronCores.

Sharding: data-parallel over batch (4) x tensor-parallel over heads (2 groups
of 8 heads). Each core computes a partial output [T, C] for one batch element
using its 8 heads; the host sums the two partials per batch element (the
"all-reduce after out_proj" done during unshard).

Single software-pipelined emission schedule (per core):
  proj(0) | attn(0)+fill[proj(1)] | attn(1)+fill[proj(2), tail(0)] |
  attn(2)+fill[proj(3), tail(1)] | attn(3)+fill[tail(2)] | tail(3)
where tail(j) = softmax-normalize + output projection + DMA for query
chunk j.  "fill" pieces are small PE matmul groups emitted between a
score matmul and its dependent AV matmul so the PE never idles while
the Scalar engine computes exp (the scalar exp stream, ~125us, is the
co-bottleneck with the PE's ~230us of matmul columns).

Per-core layouts (no on-device transposes):
  QT/KT per chunk t: [128, 4, 512] bf16  (ch%128, ch//128, tok)
  V_aug per chunk:   [128, 4, 8, 65] bf16 (key%128, tb, head, d; col 64 = 1)
  scores per (j,p,kb): PSUM [128, 2w]; w trimmed to 512-128m on the
    diagonal blocks (m = kb-4j >= 0), only the first 128 columns of a
    diagonal block need the triangular mask.
  softmax denominator rows staged in dens[32j, slot] (engine APs may only
    start at partitions {0,32,64,96}); per-j DMA compacts to [8, 512],
    reciprocal_approx_fast, then a small sel-matmul broadcasts 1/denom
    back to 64 partitions for the normalize multiply.
"""

import numpy as np
import ml_dtypes

_BF = ml_dtypes.bfloat16

import concourse.bass as bass
import concourse.bacc as bacc
import concourse.mybir as mybir
import concourse.tile as tile
from concourse import bass_utils

F32 = mybir.dt.float32
F32R = mybir.dt.float32r
BF16 = mybir.dt.bfloat16

B, T, C = 4, 2048, 1024
H, Dh = 16, 64
G = 2                 # head groups (tensor parallel)
HPG = H // G          # heads per group = 8
GC = HPG * Dh         # group channels = 512
N_CORES = 8
TC = 512              # token chunk
KB = 128              # key block
N_TC = T // TC        # 4
N_CC = C // 128       # contraction chunks over C = 8
N_GCB = GC // 128     # chan blocks in a group = 4


class _Fillers:
    """Queue of small emission pieces drained between score and AV matmuls."""

    def __init__(self):
        self.q = []
        self.i = 0

    def add(self, fn):
        self.q.append(fn)

    def remaining(self):
        return len(self.q) - self.i

    def drain(self, n):
        stop = min(self.i + n, len(self.q))
        while self.i < stop:
            self.q[self.i]()
            self.i += 1

    def drain_all(self):
        self.drain(len(self.q))


def build_program():
    nc = bacc.Bacc("TRN2", target_bir_lowering=False, debug=False)

    xT = nc.dram_tensor("xT", [C, T], BF16, kind="ExternalInput").ap()
    wq = nc.dram_tensor("wq", [C, GC], BF16, kind="ExternalInput").ap()
    wk = nc.dram_tensor("wk", [C, GC], BF16, kind="ExternalInput").ap()
    wv = nc.dram_tensor("wv", [C, GC], BF16, kind="ExternalInput").ap()
    wo = nc.dram_tensor("wo", [GC, C], BF16, kind="ExternalInput").ap()
    mask_in = nc.dram_tensor("mask", [KB, KB], BF16, kind="ExternalInput").ap()
    sel_in = nc.dram_tensor("sel", [8, 8 * Dh], BF16, kind="ExternalInput").ap()
    out = nc.dram_tensor("out", [T, C], BF16, kind="ExternalOutput").ap()

    with tile.TileContext(nc) as tc:
        with (
            tc.tile_pool(name="persist", bufs=1) as pp,
            tc.tile_pool(name="xt_pool", bufs=2) as xp,
            tc.tile_pool(name="pr_pool", bufs=3) as prp,
            tc.tile_pool(name="aot_pool", bufs=2) as aop,
            tc.tile_pool(name="ot_pool", bufs=2) as otp,
            tc.tile_pool(name="dc_pool", bufs=6) as dcp,
            tc.tile_pool(name="pj_psum", bufs=2, space="PSUM") as pjp,
            tc.tile_pool(name="sc_psum", bufs=2, space="PSUM") as scp,
            tc.tile_pool(name="av_psum", bufs=2, space="PSUM") as avp,
        ):
            wqs = pp.tile([128, N_CC, GC], BF16, tag="wq")
            wks = pp.tile([128, N_CC, GC], BF16, tag="wk")
            wvs = pp.tile([128, N_CC, GC], BF16, tag="wv")
            wos = pp.tile([128, N_GCB, C], BF16, tag="wo")
            msk = pp.tile([KB, KB], BF16, tag="msk")
            sel8 = pp.tile([8, 8 * Dh], BF16, tag="sel")
            dens = pp.tile([97, 8, TC], F32, tag="dens")
            qts = [pp.tile([128, N_GCB, TC], BF16, tag=f"qt{t}", name=f"qt{t}")
                   for t in range(N_TC)]
            kts = [pp.tile([128, N_GCB, TC], BF16, tag=f"kt{t}", name=f"kt{t}")
                   for t in range(N_TC)]
            vgs = [
                pp.tile([128, 4, HPG, Dh + 1], BF16, tag=f"vg{t}", name=f"vg{t}")
                for t in range(N_TC)
            ]

            # input DMAs, most-urgent first (proj(0) QT needs wq + xt(0));
            # interleave wq/xt chunks so the first QT group starts early
            xt0 = xp.tile([128, N_CC, TC], BF16, tag="xt", name="xt")
            for kc in range(N_CC):
                nc.scalar.dma_start(wqs[:, kc, :], wq[kc * 128:(kc + 1) * 128, :])
                nc.sync.dma_start(
                    xt0[:, kc, :], xT[kc * 128:(kc + 1) * 128, 0:TC]
                )
            for kc in range(N_CC):
                nc.scalar.dma_start(wks[:, kc, :], wk[kc * 128:(kc + 1) * 128, :])
            for kc in range(N_CC):
                nc.sync.dma_start(wvs[:, kc, :], wv[kc * 128:(kc + 1) * 128, :])
            nc.sync.dma_start(msk[:], mask_in)
            for cb in range(N_GCB):
                nc.sync.dma_start(wos[:, cb, :], wo[cb * 128:(cb + 1) * 128, :])
            nc.sync.dma_start(sel8[:], sel_in)
            for t in range(N_TC):
                nc.vector.memset(vgs[t][:, :, :, Dh:], 1.0)

            def emit_proj_pieces(t, F, xt, parts=("q", "k", "v")):
                """Append proj-chunk-t pieces (2 matmuls each) to F."""
                srcs = []
                if "q" in parts:
                    srcs.append((wqs, qts))
                if "k" in parts:
                    srcs.append((wks, kts))
                for w_s, dsts in srcs:
                    for oc in range(N_GCB):
                        box = [None]

                        def qk_piece(pc, oc=oc, w_s=w_s, dsts=dsts, box=box,
                                     t=t, xt=xt):
                            if pc == 0:
                                box[0] = pjp.tile([128, TC], F32, tag="pj", name="pj")
                            for kc in (2 * pc, 2 * pc + 1):
                                nc.tensor.matmul(
                                    box[0][:],
                                    w_s[:, kc, oc * 128:(oc + 1) * 128],
                                    xt[:, kc, :],
                                    start=(kc == 0),
                                    stop=(kc == N_CC - 1),
                                )
                            if pc == 3:
                                nc.vector.tensor_copy(dsts[t][:, oc, :], box[0][:])

                        for pc in range(4):
                            F.add(lambda pc=pc, f=qk_piece: f(pc))
                for tb in (range(4) if "v" in parts else ()):
                    box = [None]

                    def v_piece(pc, tb=tb, box=box, t=t, xt=xt):
                        if pc == 0:
                            box[0] = pjp.tile([128, GC], F32, tag="pj", name="pj")
                        for kc in (2 * pc, 2 * pc + 1):
                            nc.tensor.matmul(
                                box[0][:],
                                xt[:, kc, tb * 128:(tb + 1) * 128],
                                wvs[:, kc, :],
                                start=(kc == 0),
                                stop=(kc == N_CC - 1),
                            )
                        if pc == 3:
                            nc.vector.tensor_copy(
                                vgs[t][:, tb, :, :Dh],
                                box[0].rearrange("p (h d) -> p h d", h=HPG),
                            )

                    for pc in range(4):
                        F.add(lambda pc=pc, f=v_piece: f(pc))

            def emit_bcnorm(j, p4, aot_j, rc2):
                """Broadcast 1/denom for head-pair p4 and normalize aot."""
                for half in range(2):
                    bc = pjp.tile([Dh, TC], F32, tag="pj", name="bc")
                    nc.tensor.matmul(
                        bc[:],
                        sel8[0:2, half * Dh:(half + 1) * Dh],
                        rc2[:],
                        start=True, stop=True,
                    )
                    slc = aot_j[half * Dh:(half + 1) * Dh, p4, :]
                    nc.vector.tensor_mul(slc, slc, bc[:])

            def emit_tail_pieces(j, F, aot_j, last_bc=None):
                """Project normalized aot(j), DMA out rows 512j..512j+512.

                last_bc: deferred bc+norm closure for head-pair 3 (its
                reciprocal chain finishes just after attn(j) ends)."""
                if last_bc is not None:
                    F.add(last_bc)
                for tbl in range(4):
                    obox = [None]
                    for oc in range(2):
                        def o_piece(oc=oc, tbl=tbl, obox=obox, j=j,
                                    aot_j=aot_j):
                            if oc == 0:
                                obox[0] = otp.tile([128, C], BF16, tag="ot", name="ot")
                            ps = pjp.tile([128, TC], F32, tag="pj", name="pj")
                            for cc in range(N_GCB):
                                nc.tensor.matmul(
                                    ps[:],
                                    aot_j[:, cc, tbl * 128:(tbl + 1) * 128],
                                    wos[:, cc, oc * TC:(oc + 1) * TC],
                                    start=(cc == 0),
                                    stop=(cc == N_GCB - 1),
                                )
                            nc.vector.tensor_copy(
                                obox[0][:, oc * TC:(oc + 1) * TC], ps[:]
                            )
                            if oc == 1:
                                r0 = (4 * j + tbl) * 128
                                nc.sync.dma_start(
                                    out[r0:r0 + 128, :], obox[0][:]
                                )

                        F.add(o_piece)

            def emit_attn(j, F, Fu=None, u_deadline=10):
                """Attention for query chunk j; drains F between sc and av.

                AV matmuls run one kb behind the score matmuls (1-deep
                software pipeline) so the PE reaches av(kb) only after
                sc(kb+1)+fillers have covered the exp(kb) latency.  Fu is
                an urgent queue (KT/V projections of chunk 3) fully
                drained before iteration u_deadline."""
                aot_j = aop.tile([128, N_GCB, TC], BF16, tag="aot", name="aot")
                nkb = 4 * j + 4
                iters = 4 * nkb
                it = 0
                rec_ps = []

                def emit_av(st):
                    kb, pr, w, off, tkb, ckb, p, pav = st
                    for half in range(2):
                        nc.tensor.matmul(
                            pav[half][:, off:TC],
                            vgs[tkb][:, ckb, 2 * p + half, :],
                            pr[:, half * TC:half * TC + w],
                            start=(kb == 0),
                            stop=(kb == nkb - 1),
                        )

                def emit_fin(p, pav, defer_aot=False):
                    aot_ops = []
                    if j < 2:
                        nc.scalar.copy(
                            dens[32 * j:32 * j + 1, 2 * p, :],
                            pav[0][Dh:Dh + 1, :],
                        )
                        nc.scalar.copy(aot_j[0:Dh, p, :], pav[0][:Dh, :])
                    else:
                        nc.vector.tensor_copy(
                            dens[32 * j:32 * j + 1, 2 * p, :],
                            pav[0][Dh:Dh + 1, :],
                        )
                        aot_ops.append(lambda: nc.vector.tensor_copy(
                            aot_j[0:Dh, p, :], pav[0][:Dh, :]))
                    nc.vector.tensor_copy(
                        dens[32 * j:32 * j + 1, 2 * p + 1, :],
                        pav[1][Dh:Dh + 1, :],
                    )
                    aot_ops.append(lambda: nc.vector.tensor_copy(
                        aot_j[Dh:2 * Dh, p, :], pav[1][:Dh, :]))
                    if not defer_aot:
                        for op in aot_ops:
                            op()
                        return None
                    return aot_ops

                for p in range(4):
                    av = [
                        avp.tile([Dh + 1, TC], F32, tag="av", name=f"av{i}")
                        for i in range(2)
                    ]
                    pend = None
                    for kb in range(nkb):
                        m = kb - 4 * j
                        w = TC if m < 0 else TC - m * 128
                        off = TC - w
                        tkb, ckb = kb // 4, kb % 4
                        # halves at bank-aligned offsets 0 / 512 (a matmul
                        # output may not cross a PSUM bank boundary)
                        sc = scp.tile([128, 2 * TC], F32, tag="sc", name="sc")
                        for half in range(2):
                            p0 = half * Dh
                            nc.tensor.matmul(
                                sc[:, half * TC:half * TC + w],
                                kts[tkb][p0:p0 + Dh, p, ckb * 128:(ckb + 1) * 128],
                                qts[j][p0:p0 + Dh, p, off:TC],
                                start=True, stop=True,
                            )
                        if Fu is not None and Fu.remaining():
                            if it < u_deadline:
                                Fu.drain(-(-Fu.remaining() // (u_deadline - it)))
                            else:
                                Fu.drain_all()
                        left = iters - it
                        F.drain(-(-F.remaining() // left))  # ceil
                        pr = prp.tile([128, 2 * TC], BF16, tag="pr", name="pr")
                        sc3 = sc.rearrange("p (b c) -> p b c", b=2)[:, :, 0:w]
                        pr3 = pr.rearrange("p (b c) -> p b c", b=2)[:, :, 0:w]
                        nc.scalar.activation(
                            pr3, sc3, mybir.ActivationFunctionType.Exp,
                        )
                        if m >= 0:
                            for half in range(2):
                                pslc = pr[:, half * TC:half * TC + KB]
                                nc.gpsimd.tensor_mul(pslc, pslc, msk[:])
                        if pend is not None:
                            emit_av(pend)
                        pend = (kb, pr, w, off, tkb, ckb, p, av)
                        it += 1
                    # bc+normalize of the previous pair: PE work that does
                    # not depend on this pair's last exp — covers its wait
                    if p >= 1:
                        emit_bcnorm(j, p - 1, aot_j, rec_ps[p - 1])
                    F.drain(2)
                    emit_av(pend)
                    deferred = emit_fin(p, av, defer_aot=(j == 3 and p == 3))
                    # per-pair denominator reciprocal; its ~3us chain
                    # completes during the next pair's kb loop
                    dc2 = dcp.tile([2, TC], F32, tag="dc", name="dc")
                    nc.sync.dma_start(
                        dc2[:], dens[32 * j:32 * j + 1, 2 * p:2 * p + 2, :]
                    )
                    rf2 = dcp.tile([2, TC], F32, tag="recf", name="recf")
                    nc.vector.reciprocal_approx_fast(rf2[:], dc2[:])
                    rc2 = dcp.tile([2, TC], BF16, tag="rec", name="rec")
                    nc.vector.tensor_copy(rc2[:], rf2[:])
                    rec_ps.append(rc2)
                    if deferred:
                        for op in deferred:
                            op()

                def last_bc(j=j, aot_j=aot_j, rc2=rec_ps[3]):
                    emit_bcnorm(j, 3, aot_j, rc2)
                return aot_j, last_bc

            # ---- pipelined emission ----
            F = _Fillers()
            emit_proj_pieces(0, F, xt0)
            F.drain_all()

            aots, recs = {}, {}
            xt3 = None
            for j in range(N_TC):
                F = _Fillers()
                Fu = None
                if j + 1 < N_TC:
                    xt = xp.tile([128, N_CC, TC], BF16, tag="xt", name="xt")
                    for kc in range(N_CC):
                        nc.sync.dma_start(
                            xt[:, kc, :],
                            xT[kc * 128:(kc + 1) * 128,
                               (j + 1) * TC:(j + 2) * TC],
                        )
                    if j + 1 == 3:
                        # KT/V of chunk 3 are first needed at attn(3) kb=12;
                        # defer them into attn(3)'s urgent queue so its thin
                        # filler stream gets more PE work
                        emit_proj_pieces(3, F, xt, parts=("q",))
                        xt3 = xt
                    else:
                        emit_proj_pieces(j + 1, F, xt)
                if j == 3:
                    Fu = _Fillers()
                    emit_proj_pieces(3, Fu, xt3, parts=("k", "v"))
                if j - 1 >= 0:
                    emit_tail_pieces(j - 1, F, aots[j - 1], recs[j - 1])
                aots[j], recs[j] = emit_attn(j, F, Fu)
                F.drain_all()
                if j == N_TC - 1:
                    recs[j]()
                    recs[j] = None

            F = _Fillers()
            emit_tail_pieces(N_TC - 1, F, aots[N_TC - 1])
            F.drain_all()

    nc.compile()
    return nc


_CACHE = {}


def _make_mask():
    m = np.zeros((KB, KB), np.float32)
    for dk in range(KB):
        m[dk, dk:] = 1.0
    return m.astype(_BF)


def _make_sel():
    s = np.zeros((8, 8 * Dh), np.float32)
    for i in range(8):
        s[i, i * Dh:(i + 1) * Dh] = 1.0
    return s.astype(_BF)


def make_in_maps(x, W_qkv, W_out):
    mask = _make_mask()
    sel = _make_sel()
    in_maps = []
    for core in range(N_CORES):
        b, g = divmod(core, G)
        cs = slice(g * GC, (g + 1) * GC)
        in_maps.append({
            "xT": np.ascontiguousarray(x[b].T).astype(_BF),
            # 1/sqrt(Dh) folded into wq
            "wq": np.ascontiguousarray(W_qkv[:, cs] * 0.125).astype(_BF),
            "wk": np.ascontiguousarray(
                W_qkv[:, C + g * GC:C + (g + 1) * GC]).astype(_BF),
            "wv": np.ascontiguousarray(
                W_qkv[:, 2 * C + g * GC:2 * C + (g + 1) * GC]).astype(_BF),
            "wo": np.ascontiguousarray(W_out[cs, :]).astype(_BF),
            "mask": mask,
            "sel": sel,
        })
    return in_maps


def kernel(x, W_qkv, W_out):
    x = np.ascontiguousarray(np.asarray(x, dtype=np.float32))
    W_qkv = np.asarray(W_qkv, dtype=np.float32)
    W_out = np.asarray(W_out, dtype=np.float32)

    if "nc" not in _CACHE:
        _CACHE["nc"] = build_program()
    nc = _CACHE["nc"]

    in_maps = make_in_maps(x, W_qkv, W_out)
    res = bass_utils.run_bass_kernel_spmd(nc, in_maps, core_ids=list(range(N_CORES)))

    out = np.empty((B, T, C), np.float32)
    for b in range(B):
        out[b] = res.results[G * b]["out"].astype(np.float32)
        for g in range(1, G):
            out[b] += res.results[G * b + g]["out"].astype(np.float32)
    return out


# revision 19
# speedup vs baseline: 1.0272x; 1.0038x over previous
"""Multi-head causal attention on 8 Trainium2 NeuronCores.

Sharding: data-parallel over batch (4) x tensor-parallel over heads (2 groups
of 8 heads). Each core computes a partial output [T, C] for one batch element
using its 8 heads; the host sums the two partials per batch element (the
"all-reduce after out_proj" done during unshard).

Single software-pipelined emission schedule (per core):
  proj(0) | attn(0)+fill[proj(1)] | attn(1)+fill[proj(2), tail(0)] |
  attn(2)+fill[proj(3), tail(1)] | attn(3)+fill[tail(2)] | tail(3)
where tail(j) = softmax-normalize + output projection + DMA for query
chunk j.  "fill" pieces are small PE matmul groups emitted between a
score matmul and its dependent AV matmul so the PE never idles while
the Scalar engine computes exp (the scalar exp stream, ~125us, is the
co-bottleneck with the PE's ~230us of matmul columns).

Per-core layouts (no on-device transposes):
  QT/KT per chunk t: [128, 4, 512] bf16  (ch%128, ch//128, tok)
  V_aug per chunk:   [128, 4, 8, 65] bf16 (key%128, tb, head, d; col 64 = 1)
  scores per (j,p,kb): PSUM [128, 2w]; w trimmed to 512-128m on the
    diagonal blocks (m = kb-4j >= 0), only the first 128 columns of a
    diagonal block need the triangular mask.
  softmax denominator rows staged in dens[32j, slot] (engine APs may only
    start at partitions {0,32,64,96}); per-j DMA compacts to [8, 512],
    reciprocal_approx_fast, then a small sel-matmul broadcasts 1/denom
    back to 64 partitions for the normalize multiply.
"""

import numpy as np
import ml_dtypes

_BF = ml_dtypes.bfloat16

import concourse.bass as bass
import concourse.bacc as bacc
import concourse.mybir as mybir
import concourse.tile as tile
from concourse import bass_utils

F32 = mybir.dt.float32
F32R = mybir.dt.float32r
BF16 = mybir.dt.bfloat16

B, T, C = 4, 2048, 1024
H, Dh = 16, 64
G = 2                 # head groups (tensor parallel)
HPG = H // G          # heads per group = 8
GC = HPG * Dh         # group channels = 512
N_CORES = 8
TC = 512              # token chunk
KB = 128              # key block
N_TC = T // TC        # 4
N_CC = C // 128       # contraction chunks over C = 8
N_GCB = GC // 128     # chan blocks in a group = 4


class _Fillers:
    """Queue of small emission pieces drained between score and AV matmuls."""

    def __init__(self):
        self.q = []
        self.i = 0

    def add(self, fn):
        self.q.append(fn)

    def remaining(self):
        return len(self.q) - self.i

    def drain(self, n):
        stop = min(self.i + n, len(self.q))
        while self.i < stop:
            self.q[self.i]()
            self.i += 1

    def drain_all(self):
        self.drain(len(self.q))


def build_program():
    nc = bacc.Bacc("TRN2", target_bir_lowering=False, debug=False)

    xT = nc.dram_tensor("xT", [C, T], BF16, kind="ExternalInput").ap()
    wq = nc.dram_tensor("wq", [C, GC], BF16, kind="ExternalInput").ap()
    wk = nc.dram_tensor("wk", [C, GC], BF16, kind="ExternalInput").ap()
    wv = nc.dram_tensor("wv", [C, GC], BF16, kind="ExternalInput").ap()
    wo = nc.dram_tensor("wo", [GC, C], BF16, kind="ExternalInput").ap()
    mask_in = nc.dram_tensor("mask", [KB, KB], BF16, kind="ExternalInput").ap()
    sel_in = nc.dram_tensor("sel", [8, 8 * Dh], BF16, kind="ExternalInput").ap()
    out = nc.dram_tensor("out", [T, C], BF16, kind="ExternalOutput").ap()

    with tile.TileContext(nc) as tc:
        with (
            tc.tile_pool(name="persist", bufs=1) as pp,
            tc.tile_pool(name="xt_pool", bufs=2) as xp,
            tc.tile_pool(name="pr_pool", bufs=3) as prp,
            tc.tile_pool(name="aot_pool", bufs=2) as aop,
            tc.tile_pool(name="ot_pool", bufs=2) as otp,
            tc.tile_pool(name="dc_pool", bufs=6) as dcp,
            tc.tile_pool(name="pj_psum", bufs=2, space="PSUM") as pjp,
            tc.tile_pool(name="sc_psum", bufs=2, space="PSUM") as scp,
            tc.tile_pool(name="av_psum", bufs=2, space="PSUM") as avp,
        ):
            wqs = pp.tile([128, N_CC, GC], BF16, tag="wq")
            wks = pp.tile([128, N_CC, GC], BF16, tag="wk")
            wvs = pp.tile([128, N_CC, GC], BF16, tag="wv")
            wos = pp.tile([128, N_GCB, C], BF16, tag="wo")
            msk = pp.tile([KB, KB], BF16, tag="msk")
            sel8 = pp.tile([8, 8 * Dh], BF16, tag="sel")
            dens = pp.tile([97, 8, TC], F32, tag="dens")
            qts = [pp.tile([128, N_GCB, TC], BF16, tag=f"qt{t}", name=f"qt{t}")
                   for t in range(N_TC)]
            kts = [pp.tile([128, N_GCB, TC], BF16, tag=f"kt{t}", name=f"kt{t}")
                   for t in range(N_TC)]
            vgs = [
                pp.tile([128, 4, HPG, Dh + 1], BF16, tag=f"vg{t}", name=f"vg{t}")
                for t in range(N_TC)
            ]

            # input DMAs, most-urgent first (proj(0) QT needs wq + xt(0));
            # interleave wq/xt chunks so the first QT group starts early
            xt0 = xp.tile([128, N_CC, TC], BF16, tag="xt", name="xt")
            for kc in range(N_CC):
                nc.scalar.dma_start(wqs[:, kc, :], wq[kc * 128:(kc + 1) * 128, :])
                nc.sync.dma_start(
                    xt0[:, kc, :], xT[kc * 128:(kc + 1) * 128, 0:TC]
                )
            for kc in range(N_CC):
                nc.scalar.dma_start(wks[:, kc, :], wk[kc * 128:(kc + 1) * 128, :])
            for kc in range(N_CC):
                nc.sync.dma_start(wvs[:, kc, :], wv[kc * 128:(kc + 1) * 128, :])
            nc.sync.dma_start(msk[:], mask_in)
            for cb in range(N_GCB):
                nc.sync.dma_start(wos[:, cb, :], wo[cb * 128:(cb + 1) * 128, :])
            nc.sync.dma_start(sel8[:], sel_in)
            for t in range(N_TC):
                nc.vector.memset(vgs[t][:, :, :, Dh:], 1.0)

            def emit_proj_pieces(t, F, xt, parts=("q", "k", "v")):
                """Append proj-chunk-t pieces (2 matmuls each) to F."""
                srcs = []
                if "q" in parts:
                    srcs.append((wqs, qts))
                if "k" in parts:
                    srcs.append((wks, kts))
                for w_s, dsts in srcs:
                    for oc in range(N_GCB):
                        box = [None]

                        def qk_piece(pc, oc=oc, w_s=w_s, dsts=dsts, box=box,
                                     t=t, xt=xt):
                            if pc == 0:
                                box[0] = pjp.tile([128, TC], F32, tag="pj", name="pj")
                            for kc in (2 * pc, 2 * pc + 1):
                                nc.tensor.matmul(
                                    box[0][:],
                                    w_s[:, kc, oc * 128:(oc + 1) * 128],
                                    xt[:, kc, :],
                                    start=(kc == 0),
                                    stop=(kc == N_CC - 1),
                                )
                            if pc == 3:
                                nc.vector.tensor_copy(dsts[t][:, oc, :], box[0][:])

                        for pc in range(4):
                            F.add(lambda pc=pc, f=qk_piece: f(pc))
                for tb in (range(4) if "v" in parts else ()):
                    box = [None]

                    def v_piece(pc, tb=tb, box=box, t=t, xt=xt):
                        if pc == 0:
                            box[0] = pjp.tile([128, GC], F32, tag="pj", name="pj")
                        for kc in (2 * pc, 2 * pc + 1):
                            nc.tensor.matmul(
                                box[0][:],
                                xt[:, kc, tb * 128:(tb + 1) * 128],
                                wvs[:, kc, :],
                                start=(kc == 0),
                                stop=(kc == N_CC - 1),
                            )
                        if pc == 3:
                            nc.vector.tensor_copy(
                                vgs[t][:, tb, :, :Dh],
                                box[0].rearrange("p (h d) -> p h d", h=HPG),
                            )

                    for pc in range(4):
                        F.add(lambda pc=pc, f=v_piece: f(pc))

            def emit_bcnorm(j, p4, aot_j, rc2):
                """Broadcast 1/denom for head-pair p4 and normalize aot."""
                for half in range(2):
                    bc = pjp.tile([Dh, TC], F32, tag="pj", name="bc")
                    nc.tensor.matmul(
                        bc[:],
                        sel8[0:2, half * Dh:(half + 1) * Dh],
                        rc2[:],
                        start=True, stop=True,
                    )
                    slc = aot_j[half * Dh:(half + 1) * Dh, p4, :]
                    nc.vector.tensor_mul(slc, slc, bc[:])

            def emit_tail_pieces(j, F, aot_j, last_bc=None):
                """Project normalized aot(j), DMA out rows 512j..512j+512.

                last_bc: deferred bc+norm closure for head-pair 3 (its
                reciprocal chain finishes just after attn(j) ends)."""
                if last_bc is not None:
                    F.add(last_bc)
                for tbl in range(4):
                    obox = [None]
                    for oc in range(2):
                        def o_piece(oc=oc, tbl=tbl, obox=obox, j=j,
                                    aot_j=aot_j):
                            if oc == 0:
                                obox[0] = otp.tile([128, C], BF16, tag="ot", name="ot")
                            ps = pjp.tile([128, TC], F32, tag="pj", name="pj")
                            for cc in range(N_GCB):
                                nc.tensor.matmul(
                                    ps[:],
                                    aot_j[:, cc, tbl * 128:(tbl + 1) * 128],
                                    wos[:, cc, oc * TC:(oc + 1) * TC],
                                    start=(cc == 0),
                                    stop=(cc == N_GCB - 1),
                                )
                            oslc = obox[0][:, oc * TC:(oc + 1) * TC]
                            # final chunk: scalar is idle after the last exp —
                            # split the copies across engines and DMA each
                            # half as soon as it lands to shorten the drain
                            if j == N_TC - 1:
                                if oc == 0:
                                    nc.scalar.copy(oslc, ps[:])
                                else:
                                    nc.vector.tensor_copy(oslc, ps[:])
                                r0 = (4 * j + tbl) * 128
                                nc.sync.dma_start(
                                    out[r0:r0 + 128, oc * TC:(oc + 1) * TC],
                                    oslc,
                                )
                            else:
                                nc.vector.tensor_copy(oslc, ps[:])
                                if oc == 1:
                                    r0 = (4 * j + tbl) * 128
                                    nc.sync.dma_start(
                                        out[r0:r0 + 128, :], obox[0][:]
                                    )

                        F.add(o_piece)

            def emit_attn(j, F, Fu=None, u_deadline=10):
                """Attention for query chunk j; drains F between sc and av.

                AV matmuls run one kb behind the score matmuls (1-deep
                software pipeline) so the PE reaches av(kb) only after
                sc(kb+1)+fillers have covered the exp(kb) latency.  Fu is
                an urgent queue (KT/V projections of chunk 3) fully
                drained before iteration u_deadline."""
                aot_j = aop.tile([128, N_GCB, TC], BF16, tag="aot", name="aot")
                nkb = 4 * j + 4
                iters = 4 * nkb
                it = 0
                rec_ps = []

                def emit_av(st):
                    kb, pr, w, off, tkb, ckb, p, pav = st
                    for half in range(2):
                        nc.tensor.matmul(
                            pav[half][:, off:TC],
                            vgs[tkb][:, ckb, 2 * p + half, :],
                            pr[:, half * TC:half * TC + w],
                            start=(kb == 0),
                            stop=(kb == nkb - 1),
                        )

                def emit_fin(p, pav, defer_aot=False):
                    aot_ops = []
                    if j < 2:
                        nc.scalar.copy(
                            dens[32 * j:32 * j + 1, 2 * p, :],
                            pav[0][Dh:Dh + 1, :],
                        )
                        nc.scalar.copy(aot_j[0:Dh, p, :], pav[0][:Dh, :])
                    else:
                        nc.vector.tensor_copy(
                            dens[32 * j:32 * j + 1, 2 * p, :],
                            pav[0][Dh:Dh + 1, :],
                        )
                        aot_ops.append(lambda: nc.vector.tensor_copy(
                            aot_j[0:Dh, p, :], pav[0][:Dh, :]))
                    nc.vector.tensor_copy(
                        dens[32 * j:32 * j + 1, 2 * p + 1, :],
                        pav[1][Dh:Dh + 1, :],
                    )
                    aot_ops.append(lambda: nc.vector.tensor_copy(
                        aot_j[Dh:2 * Dh, p, :], pav[1][:Dh, :]))
                    if not defer_aot:
                        for op in aot_ops:
                            op()
                        return None
                    return aot_ops

                for p in range(4):
                    av = [
                        avp.tile([Dh + 1, TC], F32, tag="av", name=f"av{i}")
                        for i in range(2)
                    ]
                    pend = None
                    for kb in range(nkb):
                        m = kb - 4 * j
                        w = TC if m < 0 else TC - m * 128
                        off = TC - w
                        tkb, ckb = kb // 4, kb % 4
                        # halves at bank-aligned offsets 0 / 512 (a matmul
                        # output may not cross a PSUM bank boundary)
                        sc = scp.tile([128, 2 * TC], F32, tag="sc", name="sc")
                        for half in range(2):
                            p0 = half * Dh
                            nc.tensor.matmul(
                                sc[:, half * TC:half * TC + w],
                                kts[tkb][p0:p0 + Dh, p, ckb * 128:(ckb + 1) * 128],
                                qts[j][p0:p0 + Dh, p, off:TC],
                                start=True, stop=True,
                            )
                        if Fu is not None and Fu.remaining():
                            if it < u_deadline:
                                Fu.drain(-(-Fu.remaining() // (u_deadline - it)))
                            else:
                                Fu.drain_all()
                        left = iters - it
                        F.drain(-(-F.remaining() // left))  # ceil
                        pr = prp.tile([128, 2 * TC], BF16, tag="pr", name="pr")
                        sc3 = sc.rearrange("p (b c) -> p b c", b=2)[:, :, 0:w]
                        pr3 = pr.rearrange("p (b c) -> p b c", b=2)[:, :, 0:w]
                        nc.scalar.activation(
                            pr3, sc3, mybir.ActivationFunctionType.Exp,
                        )
                        if m >= 0:
                            for half in range(2):
                                pslc = pr[:, half * TC:half * TC + KB]
                                nc.gpsimd.tensor_mul(pslc, pslc, msk[:])
                        if pend is not None:
                            emit_av(pend)
                        pend = (kb, pr, w, off, tkb, ckb, p, av)
                        it += 1
                    # bc+normalize of the previous pair: PE work that does
                    # not depend on this pair's last exp — covers its wait
                    if p >= 1:
                        emit_bcnorm(j, p - 1, aot_j, rec_ps[p - 1])
                    F.drain(2)
                    emit_av(pend)
                    deferred = emit_fin(p, av, defer_aot=(j == 3 and p == 3))
                    # per-pair denominator reciprocal; its ~3us chain
                    # completes during the next pair's kb loop
                    dc2 = dcp.tile([2, TC], F32, tag="dc", name="dc")
                    nc.sync.dma_start(
                        dc2[:], dens[32 * j:32 * j + 1, 2 * p:2 * p + 2, :]
                    )
                    rf2 = dcp.tile([2, TC], F32, tag="recf", name="recf")
                    nc.vector.reciprocal_approx_fast(rf2[:], dc2[:])
                    rc2 = dcp.tile([2, TC], BF16, tag="rec", name="rec")
                    nc.vector.tensor_copy(rc2[:], rf2[:])
                    rec_ps.append(rc2)
                    if deferred:
                        for op in deferred:
                            op()

                def last_bc(j=j, aot_j=aot_j, rc2=rec_ps[3]):
                    emit_bcnorm(j, 3, aot_j, rc2)
                return aot_j, last_bc

            # ---- pipelined emission ----
            F = _Fillers()
            emit_proj_pieces(0, F, xt0)
            F.drain_all()

            aots, recs = {}, {}
            xt3 = None
            for j in range(N_TC):
                F = _Fillers()
                Fu = None
                if j + 1 < N_TC:
                    xt = xp.tile([128, N_CC, TC], BF16, tag="xt", name="xt")
                    for kc in range(N_CC):
                        nc.sync.dma_start(
                            xt[:, kc, :],
                            xT[kc * 128:(kc + 1) * 128,
                               (j + 1) * TC:(j + 2) * TC],
                        )
                    if j + 1 == 3:
                        # KT/V of chunk 3 are first needed at attn(3) kb=12;
                        # defer them into attn(3)'s urgent queue so its thin
                        # filler stream gets more PE work
                        emit_proj_pieces(3, F, xt, parts=("q",))
                        xt3 = xt
                    else:
                        emit_proj_pieces(j + 1, F, xt)
                if j == 3:
                    Fu = _Fillers()
                    emit_proj_pieces(3, Fu, xt3, parts=("k", "v"))
                if j - 1 >= 0:
                    emit_tail_pieces(j - 1, F, aots[j - 1], recs[j - 1])
                aots[j], recs[j] = emit_attn(j, F, Fu)
                F.drain_all()
                if j == N_TC - 1:
                    recs[j]()
                    recs[j] = None

            F = _Fillers()
            emit_tail_pieces(N_TC - 1, F, aots[N_TC - 1])
            F.drain_all()

    nc.compile()
    return nc


_CACHE = {}


def _make_mask():
    m = np.zeros((KB, KB), np.float32)
    for dk in range(KB):
        m[dk, dk:] = 1.0
    return m.astype(_BF)


def _make_sel():
    s = np.zeros((8, 8 * Dh), np.float32)
    for i in range(8):
        s[i, i * Dh:(i + 1) * Dh] = 1.0
    return s.astype(_BF)


def make_in_maps(x, W_qkv, W_out):
    mask = _make_mask()
    sel = _make_sel()
    in_maps = []
    for core in range(N_CORES):
        b, g = divmod(core, G)
        cs = slice(g * GC, (g + 1) * GC)
        in_maps.append({
            "xT": np.ascontiguousarray(x[b].T).astype(_BF),
            # 1/sqrt(Dh) folded into wq
            "wq": np.ascontiguousarray(W_qkv[:, cs] * 0.125).astype(_BF),
            "wk": np.ascontiguousarray(
                W_qkv[:, C + g * GC:C + (g + 1) * GC]).astype(_BF),
            "wv": np.ascontiguousarray(
                W_qkv[:, 2 * C + g * GC:2 * C + (g + 1) * GC]).astype(_BF),
            "wo": np.ascontiguousarray(W_out[cs, :]).astype(_BF),
            "mask": mask,
            "sel": sel,
        })
    return in_maps


def kernel(x, W_qkv, W_out):
    x = np.ascontiguousarray(np.asarray(x, dtype=np.float32))
    W_qkv = np.asarray(W_qkv, dtype=np.float32)
    W_out = np.asarray(W_out, dtype=np.float32)

    if "nc" not in _CACHE:
        _CACHE["nc"] = build_program()
    nc = _CACHE["nc"]

    in_maps = make_in_maps(x, W_qkv, W_out)
    res = bass_utils.run_bass_kernel_spmd(nc, in_maps, core_ids=list(range(N_CORES)))

    out = np.empty((B, T, C), np.float32)
    for b in range(B):
        out[b] = res.results[G * b]["out"].astype(np.float32)
        for g in range(1, G):
            out[b] += res.results[G * b + g]["out"].astype(np.float32)
    return out
